# revision 11
# baseline (speedup 1.0000x reference)
"""BiMamba2Dv2 Trainium2 kernel (v2).

8 cores = 4 batches x 2 scan directions; each core runs both Mamba stages
for its (batch, dir) in feature-on-partition layout [C|Di, L].

v2 engine mapping (vs baseline):
- selective-scan runs in HALVES (L/2=1152) per (n, block): DVE
  tensor_tensor_scan at ~2.1ns/col with rotating buffers (no drain gaps),
  chained across halves via a [128, 3*16] last-state gather.
- B/C broadcast loads go through the SP hardware DMA queue (frees POOL).
- E=exp(A_n*delta) on ScalarE; delta via one Softplus pass (PSUM src);
  conv as 3 shifted diagonal matmuls on TensorE + Silu on ScalarE;
  z-gate silu on ScalarE straight from PSUM; sz/y stay in SBUF.
- X=du*B on DVE (bf16 2x); hm=h*C split DVE/POOL by state index.
- n-summation via identity matmuls into bank-aligned PSUM tiles
  ([128,1024] x3 + [128,384] tail).
"""

import sys

for _p in ("/opt/trn_rl_repo", "/root/.axon_site/_ro/trn_rl_repo"):
    if _p not in sys.path:
        sys.path.insert(0, _p)

import numpy as np
import ml_dtypes

import concourse.bass as bass
import concourse.bacc as bacc
import concourse.tile as tile
from concourse import mybir
from concourse.bass_utils import run_bass_kernel_spmd

BF16 = ml_dtypes.bfloat16

B, H, W = 4, 48, 48
C = 192
DI = 384
NB = 3             # d-blocks of 128
NST = 16           # state dim
RNK = 12           # dt rank
L = H * W          # 2304
HL = L // 2        # 1152
NCORES = 8
T_TILES = [(0, 512), (512, 512), (1024, 512), (1536, 512), (2048, 256)]
# hm-mul ops on POOL for these states (tunable load-balance knob)
POOL_HM_N = {2, 5, 8, 11, 14}
POOL_X_N = {3, 9, 15}

F32 = mybir.dt.float32
BF = mybir.dt.bfloat16
MUL = mybir.AluOpType.mult
ADD = mybir.AluOpType.add
SUB = mybir.AluOpType.subtract
AFT = mybir.ActivationFunctionType


def _ap(t, free_pairs, off, parts=None):
    part_pair = t.ap[0] if parts is None else parts
    return bass.AP(tensor=t.tensor, offset=t.offset + off, ap=[part_pair] + free_pairs)


def _emit_stage(nc, pools, Wt, u_bf, sfx, A_vals, partial_dram, bc_dram):
    big, med, scr, ps = pools["big"], pools["med"], pools["scr"], pools["ps"]

    w_in = Wt[f"win_{sfx}"]
    w_out = Wt[f"wout_{sfx}"]
    w_xp = Wt[f"wxp_{sfx}"]
    w_dt = Wt[f"wdt_{sfx}"]
    convd = Wt[f"convd_{sfx}"]     # [NB][3] diag matrices on SBUF (bf16)
    convb = Wt[f"convb_{sfx}"]
    dtb = Wt[f"dtb_{sfx}"]
    dvec = Wt[f"dvec_{sfx}"]
    ident = Wt["ident"]

    # ---------------- P1: in_proj -> xh (bf16) and sz = silu(z) ----------------
    xh = med.tile([128, NB * L], BF, tag="medA", name=f"xh_{sfx}")
    sz = med.tile([128, NB * L], BF, tag="medB", name=f"sz_{sfx}")
    for m in range(6):
        for (t0, tsz) in T_TILES:
            pt = ps.tile([128, 512], F32, tag="ps", name=f"p1_{sfx}", bufs=1)
            for k in range(2):
                nc.tensor.matmul(
                    pt[:, :tsz],
                    w_in[k][:, m * 128:(m + 1) * 128],
                    u_bf[k][:, t0:t0 + tsz],
                    start=(k == 0), stop=(k == 1))
            if m < 3:
                nc.scalar.activation(xh[:, m * L + t0: m * L + t0 + tsz],
                                     pt[:, :tsz], AFT.Copy)
            else:
                mm = m - 3
                nc.scalar.activation(sz[:, mm * L + t0: mm * L + t0 + tsz],
                                     pt[:, :tsz], AFT.Silu)

    # ---------------- conv via 3 shifted diag matmuls + Silu ----------------
    xc = med.tile([128, NB * L], BF, tag="medC", name=f"xc_{sfx}")
    for b in range(NB):
        xb_off = b * L
        for (t0, tsz) in T_TILES:
            pt = ps.tile([128, 512], F32, tag="ps", name=f"pcv_{sfx}", bufs=1)
            # tap j multiplies xh[t - (2 - j)]; j=2 (no shift) first, start=True
            nc.tensor.matmul(
                pt[:, :tsz], convd[b][2],
                xh[:, xb_off + t0: xb_off + t0 + tsz],
                start=True, stop=False)
            for j in (1, 0):
                sh = 2 - j
                s0 = t0 - sh
                if s0 >= 0:
                    nc.tensor.matmul(
                        pt[:, :tsz], convd[b][j],
                        xh[:, xb_off + s0: xb_off + s0 + tsz],
                        start=False, stop=(j == 0))
                else:
                    # chunk 0: shifted tap covers out cols sh.. only
                    nc.tensor.matmul(
                        pt[:, sh:tsz], convd[b][j],
                        xh[:, xb_off: xb_off + tsz - sh],
                        start=False, stop=False)
            if t0 == 0:
                nc.tensor.matmul(
                    pt[:, :tsz], Wt["zero128"],
                    xh[:, xb_off + t0: xb_off + t0 + tsz],
                    start=False, stop=True)
            nc.scalar.activation(xc[:, xb_off + t0: xb_off + t0 + tsz],
                                 pt[:, :tsz], AFT.Silu, bias=convb[b])

    # ---------------- x_proj -> dt rows + B/C rows ----------------
    xdbl = scr.tile([12, L], BF, tag="xdbl", name=f"xdbl_{sfx}", bufs=1)
    bcbf = scr.tile([32, L], BF, tag="bcbf", name=f"bcbf_{sfx}", bufs=1)
    for (t0, tsz) in T_TILES:
        pt = ps.tile([12, 512], F32, tag="ps", name=f"pxp_{sfx}", bufs=1)
        pb = ps.tile([32, 512], F32, tag="ps2", name=f"pxb_{sfx}", bufs=1)
        for k in range(NB):
            nc.tensor.matmul(
                pt[:, :tsz], w_xp[k][:, 0:RNK],
                xc[:, k * L + t0: k * L + t0 + tsz],
                start=(k == 0), stop=(k == NB - 1))
            nc.tensor.matmul(
                pb[:, :tsz], w_xp[k][:, RNK:44],
                xc[:, k * L + t0: k * L + t0 + tsz],
                start=(k == 0), stop=(k == NB - 1))
        nc.vector.tensor_copy(xdbl[:, t0:t0 + tsz], pt[:, :tsz])
        nc.scalar.activation(bcbf[:, t0:t0 + tsz], pb[:, :tsz], AFT.Copy)
    # B/C rows -> DRAM (for partition-broadcast loads)
    nc.sync.dma_start(out=bc_dram[:, :], in_=bcbf)

    # ---------------- dt_proj + softplus -> delta (f32) ----------------
    delta = big.tile([128, NB * L], F32, tag="bigA", name=f"delta_{sfx}")
    for m in range(NB):
        for (t0, tsz) in T_TILES:
            pt = ps.tile([128, 512], F32, tag="ps", name=f"pdt_{sfx}", bufs=1)
            nc.tensor.matmul(
                pt[:, :tsz], w_dt[:, m * 128:(m + 1) * 128],
                xdbl[:, t0:t0 + tsz], start=True, stop=True)
            nc.scalar.activation(delta[:, m * L + t0: m * L + t0 + tsz],
                                 pt[:, :tsz], AFT.Exp, bias=dtb[m])

    for m in range(NB):
        nc.scalar.activation(delta[:, m * L:(m + 1) * L], delta[:, m * L:(m + 1) * L],
                             AFT.Ln, bias=Wt["ones_col"])

    # du = delta * xc (mixed f32*bf16, 1x)
    du = med.tile([128, NB * L], BF, tag="medE", name=f"du_{sfx}")
    nc.vector.tensor_tensor(out=du, in0=delta, in1=xc, op=MUL)

    # ---------------- P2: selective scan (halves, n inner) ----------------
    y = med.tile([128, NB * L], BF, tag="medA", name=f"y_{sfx}")  # reuses xh slot
    hlast = scr.tile([128, NB * NST], BF, tag="hlast", name=f"hl_{sfx}", bufs=1)
    for hf in range(2):
        q0 = hf * HL
        ypb = [pools["ps_big"].tile([128, 1024], F32, tag=f"ypb{b}", name=f"ypb{b}_{sfx}")
               for b in range(NB)]
        ypt = ps.tile([128, NB * 128], F32, tag="ps2", name=f"ypt_{sfx}", bufs=1)
        for n in range(NST):
            Eb = []
            for b in range(NB):
                E = big.tile([128, HL], F32, tag="E", name=f"E{b}_{sfx}", bufs=3)
                nc.scalar.activation(
                    E, delta[:, b * L + q0: b * L + q0 + HL],
                    AFT.Exp, scale=float(A_vals[n]))
                Eb.append(E)
            bcB = scr.tile([128, HL], BF, tag="bcB", name=f"bcB_{sfx}", bufs=2)
            bcC = scr.tile([128, HL], BF, tag="bcC", name=f"bcC_{sfx}", bufs=2)
            nc.sync.dma_start(
                out=bcB, in_=bc_dram.ap()[n:n + 1, q0:q0 + HL].partition_broadcast(128))
            nc.sync.dma_start(
                out=bcC, in_=bc_dram.ap()[NST + n:NST + n + 1, q0:q0 + HL].partition_broadcast(128))
            X = scr.tile([128, NB * HL], BF, tag="X", name=f"X_{sfx}", bufs=2)
            if n in POOL_X_N:
                nc.gpsimd.tensor_tensor(
                    out=_ap(X, [[HL, NB], [1, HL]], 0),
                    in0=_ap(du, [[L, NB], [1, HL]], q0),
                    in1=_ap(bcB, [[0, NB], [1, HL]], 0), op=MUL)
            else:
                nc.vector.tensor_tensor(
                    out=_ap(X, [[HL, NB], [1, HL]], 0),
                    in0=_ap(du, [[L, NB], [1, HL]], q0),
                    in1=_ap(bcB, [[0, NB], [1, HL]], 0), op=MUL)
            h = scr.tile([128, NB * HL], BF, tag="h", name=f"h_{sfx}", bufs=2)
            for b in range(NB):
                init = 0.0 if hf == 0 else hlast[:, n * NB + b: n * NB + b + 1]
                nc.vector.tensor_tensor_scan(
                    h[:, b * HL:(b + 1) * HL],
                    Eb[b],
                    X[:, b * HL:(b + 1) * HL],
                    init, MUL, ADD)
            if hf == 0:
                nc.vector.tensor_copy(
                    hlast[:, n * NB: n * NB + NB],
                    _ap(h, [[HL, NB], [1, 1]], HL - 1))
            hm = scr.tile([128, NB * HL], BF, tag="X", name=f"hm_{sfx}", bufs=2)
            eng = nc.gpsimd if n in POOL_HM_N else nc.vector
            eng.tensor_tensor(
                out=_ap(hm, [[HL, NB], [1, HL]], 0),
                in0=_ap(h, [[HL, NB], [1, HL]], 0),
                in1=_ap(bcC, [[0, NB], [1, HL]], 0), op=MUL)
            st = (n == 0)
            sp = (n == NST - 1)
            for b in range(NB):
                nc.tensor.matmul(ypb[b][:, 0:512], ident, hm[:, b * HL: b * HL + 512],
                                 start=st, stop=sp)
                nc.tensor.matmul(ypb[b][:, 512:1024], ident, hm[:, b * HL + 512: b * HL + 1024],
                                 start=st, stop=sp)
                nc.tensor.matmul(ypt[:, b * 128:(b + 1) * 128], ident,
                                 hm[:, b * HL + 1024: (b + 1) * HL],
                                 start=st, stop=sp)
        # y = ypsum + xc * D   (per block, this half)
        for b in range(NB):
            nc.vector.scalar_tensor_tensor(
                y[:, b * L + q0: b * L + q0 + 1024],
                xc[:, b * L + q0: b * L + q0 + 1024],
                dvec[b], ypb[b][:, :], MUL, ADD)
            nc.vector.scalar_tensor_tensor(
                y[:, b * L + q0 + 1024: b * L + q0 + HL],
                xc[:, b * L + q0 + 1024: b * L + q0 + HL],
                dvec[b], ypt[:, b * 128:(b + 1) * 128], MUL, ADD)

    # ---------------- P3: gate + out_proj ----------------
    yg = med.tile([128, NB * L], BF, tag="medC", name=f"yg_{sfx}")  # reuses xc slot
    nc.vector.tensor_tensor(out=yg, in0=y, in1=sz, op=MUL)
    for m in range(2):
        msz = 128 if m == 0 else 64
        for (t0, tsz) in T_TILES:
            pt = ps.tile([128, 512], F32, tag="ps", name=f"pout_{sfx}", bufs=1)
            for k in range(NB):
                nc.tensor.matmul(
                    pt[:msz, :tsz],
                    w_out[k][:, m * 128: m * 128 + msz],
                    yg[:, k * L + t0: k * L + t0 + tsz],
                    start=(k == 0), stop=(k == NB - 1))
            stg = scr.tile([128, 512], F32, tag="stg", name=f"stg_{sfx}", bufs=1)
            nc.scalar.activation(stg[:msz, :tsz], pt[:msz, :tsz], AFT.Copy)
            nc.sync.dma_start(out=partial_dram.ap()[m * 128: m * 128 + msz, t0:t0 + tsz],
                              in_=stg[:msz, :tsz])


def build_nc(A_vals):
    nc = bacc.Bacc("TRN2", target_bir_lowering=False, debug=False,
                   enable_asserts=False, num_devices=NCORES)

    u0_bf = nc.dram_tensor("u0_bf", [C, L], BF, kind="ExternalInput")
    xres = nc.dram_tensor("xres", [C, L], F32, kind="ExternalInput")
    mask = nc.dram_tensor("mask", [128, 1], F32, kind="ExternalInput")
    maskinv = nc.dram_tensor("maskinv", [128, 1], F32, kind="ExternalInput")
    normw = nc.dram_tensor("normw", [C, 1], F32, kind="ExternalInput")
    normb = nc.dram_tensor("normb", [C, 1], F32, kind="ExternalInput")
    ident_in = nc.dram_tensor("ident", [128, 128], BF, kind="ExternalInput")
    wdecl = {}
    for s in ("a", "b"):
        wdecl[f"win_{s}"] = nc.dram_tensor(f"win_{s}", [C, 2 * DI], BF, kind="ExternalInput")
        wdecl[f"wout_{s}"] = nc.dram_tensor(f"wout_{s}", [DI, C], BF, kind="ExternalInput")
        wdecl[f"wxp_{s}"] = nc.dram_tensor(f"wxp_{s}", [DI, 44], BF, kind="ExternalInput")
        wdecl[f"wdt_{s}"] = nc.dram_tensor(f"wdt_{s}", [RNK, DI], BF, kind="ExternalInput")
        wdecl[f"convd_{s}"] = nc.dram_tensor(f"convd_{s}", [NB * 3 * 128, 128], BF, kind="ExternalInput")
        wdecl[f"convb_{s}"] = nc.dram_tensor(f"convb_{s}", [DI, 1], F32, kind="ExternalInput")
        wdecl[f"dtb_{s}"] = nc.dram_tensor(f"dtb_{s}", [DI, 1], F32, kind="ExternalInput")
        wdecl[f"dvec_{s}"] = nc.dram_tensor(f"dvec_{s}", [DI, 1], F32, kind="ExternalInput")
    out_full = nc.dram_tensor("out_full", [C, L], F32, kind="ExternalOutput")

    partial_a = nc.dram_tensor("partial_a", [C, L], F32)
    ssum_a = nc.dram_tensor("ssum_a", [C, L], F32)
    partial_b = nc.dram_tensor("partial_b", [C, L], F32)
    ssum_b = nc.dram_tensor("ssum_b", [C, L], F32)
    bc_dram_a = nc.dram_tensor("bc_dram_a", [32, L], BF)
    bc_dram_b = nc.dram_tensor("bc_dram_b", [32, L], BF)
    stats_dram = nc.dram_tensor("stats_dram", [2, L], F32)

    groups = [[b, b + 4] for b in range(B)]

    import contextlib
    with contextlib.ExitStack() as ctx:
        tc = ctx.enter_context(tile.TileContext(nc))
        pools = {
            "w": ctx.enter_context(tc.tile_pool(name="w", bufs=1)),
            "big": ctx.enter_context(tc.tile_pool(name="big", bufs=1)),
            "med": ctx.enter_context(tc.tile_pool(name="med", bufs=1)),
            "scr": ctx.enter_context(tc.tile_pool(name="scr", bufs=2)),
            "glue": ctx.enter_context(tc.tile_pool(name="glue", bufs=1)),
            "ps": ctx.enter_context(tc.tile_pool(name="ps", bufs=2, space="PSUM")),
            "ps_big": ctx.enter_context(tc.tile_pool(name="ps_big", bufs=1, space="PSUM")),
        }
        wp = pools["w"]

        Wt = {}
        for s in ("a", "b"):
            t1 = wp.tile([128, 2 * DI], BF, tag=f"win0{s}", name=f"win0{s}")
            t2 = wp.tile([64, 2 * DI], BF, tag=f"win1{s}", name=f"win1{s}")
            nc.sync.dma_start(out=t1, in_=wdecl[f"win_{s}"].ap()[0:128, :])
            nc.sync.dma_start(out=t2, in_=wdecl[f"win_{s}"].ap()[128:192, :])
            Wt[f"win_{s}"] = [t1, t2]
            Wt[f"wout_{s}"] = []
            for k in range(NB):
                t = wp.tile([128, C], BF, tag=f"wout{k}{s}", name=f"wout{k}{s}")
                nc.sync.dma_start(out=t, in_=wdecl[f"wout_{s}"].ap()[k * 128:(k + 1) * 128, :])
                Wt[f"wout_{s}"].append(t)
            Wt[f"wxp_{s}"] = []
            for k in range(NB):
                t = wp.tile([128, 44], BF, tag=f"wxp{k}{s}", name=f"wxp{k}{s}")
                nc.sync.dma_start(out=t, in_=wdecl[f"wxp_{s}"].ap()[k * 128:(k + 1) * 128, :])
                Wt[f"wxp_{s}"].append(t)
            t = wp.tile([RNK, DI], BF, tag=f"wdt{s}", name=f"wdt{s}")
            nc.sync.dma_start(out=t, in_=wdecl[f"wdt_{s}"].ap()[:, :])
            Wt[f"wdt_{s}"] = t
            # conv diag matrices [NB][3] each [128,128] bf16
            Wt[f"convd_{s}"] = []
            for k in range(NB):
                taps = []
                for j in range(3):
                    t = wp.tile([128, 128], BF, tag=f"cvd{k}{j}{s}", name=f"cvd{k}{j}{s}")
                    off = (k * 3 + j) * 128
                    nc.sync.dma_start(out=t, in_=wdecl[f"convd_{s}"].ap()[off:off + 128, :])
                    taps.append(t)
                Wt[f"convd_{s}"].append(taps)
            for nm in ("convb", "dtb", "dvec"):
                lst = []
                for k in range(NB):
                    t = wp.tile([128, 1], F32, tag=f"{nm}{k}{s}", name=f"{nm}{k}{s}")
                    nc.sync.dma_start(out=t, in_=wdecl[f"{nm}_{s}"].ap()[k * 128:(k + 1) * 128, :])
                    lst.append(t)
                Wt[f"{nm}_{s}"] = lst
        idt = wp.tile([128, 128], BF, tag="ident", name="ident_t")
        nc.sync.dma_start(out=idt, in_=ident_in.ap()[:, :])
        Wt["ident"] = idt
        z128 = wp.tile([128, 128], BF, tag="zero128", name="zero128")
        nc.vector.memset(z128, 0.0)
        Wt["zero128"] = z128
        nw = [wp.tile([128, 1], F32, tag="nw0", name="nw0"),
              wp.tile([64, 1], F32, tag="nw1", name="nw1")]
        nb_ = [wp.tile([128, 1], F32, tag="nb0", name="nb0"),
               wp.tile([64, 1], F32, tag="nb1", name="nb1")]
        nc.sync.dma_start(out=nw[0], in_=normw.ap()[0:128, :])
        nc.sync.dma_start(out=nw[1], in_=normw.ap()[128:192, :])
        nc.sync.dma_start(out=nb_[0], in_=normb.ap()[0:128, :])
        nc.sync.dma_start(out=nb_[1], in_=normb.ap()[128:192, :])
        msk = wp.tile([128, 1], F32, tag="msk", name="msk")
        mskv = wp.tile([128, 1], F32, tag="mskv", name="mskv")
        nc.sync.dma_start(out=msk, in_=mask.ap()[:, :])
        nc.sync.dma_start(out=mskv, in_=maskinv.ap()[:, :])
        ones_a = wp.tile([128, 1], F32, tag="ones_a", name="ones_a")
        ones_b = wp.tile([64, 1], F32, tag="ones_b", name="ones_b")
        nc.vector.memset(ones_a, 1.0)
        nc.vector.memset(ones_b, 1.0)
        Wt["ones_col"] = ones_a

        uA = [wp.tile([128, L], BF, tag="uin0", name="uA0"),
              wp.tile([64, L], BF, tag="uin1", name="uA1")]
        nc.sync.dma_start(out=uA[0], in_=u0_bf.ap()[0:128, :])
        nc.sync.dma_start(out=uA[1], in_=u0_bf.ap()[128:192, :])

        _emit_stage(nc, pools, Wt, uA, "a", A_vals, partial_a, bc_dram_a)

        nc.gpsimd.collective_compute(
            "AllReduce", ADD, replica_groups=groups,
            ins=[partial_a.ap().opt()], outs=[ssum_a.ap().opt()])

        # ---------------- glue ----------------
        gl = pools["glue"]
        big = pools["big"]
        med = pools["med"]
        # packed [128, 2L]: cols 0:L = channels 0..127, cols L:2L (rows 0:64) = 128..191
        st = big.tile([128, 2 * L], F32, tag="bigB", name="st_g")
        fl = big.tile([128, 2 * L], BF, tag="bigC", name="fl_g")
        res = med.tile([128, 2 * L], BF, tag="medB", name="res_g")
        sq = big.tile([128, 2 * L], F32, tag="bigA", name="sq_g")
        rA = gl.tile([1, L], F32, tag="rA", name="rA_g")
        rB = gl.tile([1, L], F32, tag="rA", name="rB_g")
        epst = gl.tile([1, 1], F32, tag="epst", name="epst_g")
        ssb = med.tile([128, 2 * L], F32, tag="medA", name="ssb_g")
        nc.sync.dma_start(out=ssb[:, 0:L], in_=ssum_a.ap()[0:128, :])
        nc.sync.dma_start(out=ssb[0:64, L:2 * L], in_=ssum_a.ap()[128:192, :])
        for p in range(2):
            psz = 128 if p == 0 else 64
            co = p * L
            # permuted straight view (DVE) & flipped view (POOL) in parallel
            nc.vector.tensor_copy(
                _ap(st, [[48, 48], [1, 48]], co, parts=[st.ap[0][0], psz]),
                _ap(ssb, [[1, 48], [48, 48]], co, parts=[ssb.ap[0][0], psz]))
            nc.gpsimd.tensor_copy(
                _ap(fl, [[48, 48], [1, 48]], co, parts=[fl.ap[0][0], psz]),
                _ap(ssb, [[-1, 48], [-48, 48]], co + L - 1, parts=[ssb.ap[0][0], psz]))
            nc.gpsimd.dma_start(out=res[0:psz, co:co + L], in_=xres.ap()[p * 128:p * 128 + psz, :])
            # select: st = st*maskinv + fl*mask
            nc.vector.tensor_scalar(out=fl[0:psz, co:co + L], in0=fl[0:psz, co:co + L],
                                    scalar1=msk[:psz, :], scalar2=None, op0=MUL)
            nc.vector.scalar_tensor_tensor(
                st[0:psz, co:co + L], st[0:psz, co:co + L], mskv[:psz, :],
                fl[0:psz, co:co + L], MUL, ADD)

        # pass 1: mean over channels via ones-matmul
        for (t0, tsz) in T_TILES:
            p1 = pools["ps"].tile([1, 512], F32, tag="ps", name="lnp1", bufs=1)
            for p in range(2):
                one = ones_a if p == 0 else ones_b
                nc.tensor.matmul(p1[:, :tsz], one,
                                 st[0:(128 if p == 0 else 64), p * L + t0: p * L + t0 + tsz],
                                 start=(p == 0), stop=(p == 1))
            nc.vector.tensor_copy(rA[:, t0:t0 + tsz], p1[:, :tsz])
        nc.vector.tensor_scalar(out=rA, in0=rA, scalar1=1.0 / C, scalar2=None, op0=MUL)
        nc.sync.dma_start(out=stats_dram[0:1, :], in_=rA)
        mbc = big.tile([128, L], F32, tag="bigC", name="mbc_g")
        nc.sync.dma_start(out=mbc, in_=stats_dram.ap()[0:1, :].partition_broadcast(128))
        # center x, square, pass 2: variance
        for p in range(2):
            psz = 128 if p == 0 else 64
            co = p * L
            nc.vector.tensor_tensor(out=st[0:psz, co:co + L], in0=st[0:psz, co:co + L],
                                    in1=mbc[0:psz, :], op=SUB)
            nc.scalar.activation(sq[0:psz, co:co + L], st[0:psz, co:co + L], AFT.Square)
        for (t0, tsz) in T_TILES:
            p2 = pools["ps"].tile([1, 512], F32, tag="ps", name="lnp2", bufs=1)
            for p in range(2):
                one = ones_a if p == 0 else ones_b
                nc.tensor.matmul(p2[:, :tsz], one,
                                 sq[0:(128 if p == 0 else 64), p * L + t0: p * L + t0 + tsz],
                                 start=(p == 0), stop=(p == 1))
            nc.vector.tensor_copy(rB[:, t0:t0 + tsz], p2[:, :tsz])
        nc.vector.tensor_scalar(out=rB, in0=rB, scalar1=1.0 / C, scalar2=None, op0=MUL)
        nc.vector.memset(epst, 1e-5)
        nc.scalar.activation(rB, rB, AFT.Sqrt, bias=epst)
        nc.vector.reciprocal(rB, rB)
        nc.sync.dma_start(out=stats_dram[1:2, :], in_=rB)
        rbc = big.tile([128, L], F32, tag="bigA", name="rbc_g")
        nc.sync.dma_start(out=rbc, in_=stats_dram.ap()[1:2, :].partition_broadcast(128))
        uB = [wp.tile([128, L], BF, tag="uin0", name="uB0"),
              wp.tile([64, L], BF, tag="uin1", name="uB1")]
        for p in range(2):
            psz = 128 if p == 0 else 64
            co = p * L
            sl = st[0:psz, co:co + L]
            nc.vector.tensor_tensor(out=sl, in0=sl, in1=rbc[0:psz, :], op=MUL)
            nc.vector.scalar_tensor_tensor(sl, sl, nw[p], res[0:psz, co:co + L], MUL, ADD)
            nc.vector.tensor_scalar(out=sl, in0=sl, scalar1=nb_[p], scalar2=None, op0=ADD)
            nc.vector.tensor_copy(uB[p], sl)

        _emit_stage(nc, pools, Wt, uB, "b", A_vals, partial_b, bc_dram_b)

        nc.gpsimd.collective_compute(
            "AllReduce", ADD, replica_groups=groups,
            ins=[partial_b.ap().opt()], outs=[ssum_b.ap().opt()])

        ob = big.tile([128, 2 * L], F32, tag="bigB", name="ob_g")
        nc.sync.dma_start(out=ob[:, 0:L], in_=ssum_b.ap()[0:128, :])
        nc.sync.dma_start(out=ob[0:64, L:2 * L], in_=ssum_b.ap()[128:192, :])
        nc.sync.dma_start(out=out_full[0:128, :], in_=ob[:, 0:L])
        nc.sync.dma_start(out=out_full[128:192, :], in_=ob[0:64, L:2 * L])

    nc.compile()
    return nc


_CACHE = {}


def make_in_maps(inputs):
    x = np.asarray(inputs["x"], np.float32)
    in_maps = []
    for core in range(NCORES):
        b, dr = core % 4, core // 4
        xw = x[b].transpose(1, 0, 2).reshape(L, C).T.copy()
        xh_ = x[b].reshape(L, C).T.copy()
        if dr == 1:
            xw = xw[:, ::-1].copy()
            xh_ = xh_[:, ::-1].copy()
        m = {
            "u0_bf": xw.astype(BF16),
            "xres": xh_.astype(np.float32),
            "mask": np.full((128, 1), float(dr), np.float32),
            "maskinv": np.full((128, 1), 1.0 - float(dr), np.float32),
            "normw": np.asarray(inputs["norm_w"], np.float32).reshape(C, 1).copy(),
            "normb": np.asarray(inputs["norm_b"], np.float32).reshape(C, 1).copy(),
            "ident": np.eye(128, dtype=BF16),
        }
        for s, i in (("a", dr), ("b", 2 + dr)):
            m[f"win_{s}"] = np.asarray(inputs["in_proj_w"][i], np.float32).T.copy().astype(BF16)
            m[f"wout_{s}"] = np.asarray(inputs["out_proj_w"][i], np.float32).T.copy().astype(BF16)
            m[f"wxp_{s}"] = np.asarray(inputs["x_proj_w"][i], np.float32).T.copy().astype(BF16)
            m[f"wdt_{s}"] = np.asarray(inputs["dt_proj_w"][i], np.float32).T.copy().astype(BF16)
            cw = np.asarray(inputs["conv_w"][i], np.float32)  # [DI, 3]
            cd = np.zeros((NB * 3 * 128, 128), np.float32)
            for k in range(NB):
                for j in range(3):
                    off = (k * 3 + j) * 128
                    cd[off:off + 128, :] = np.diag(cw[k * 128:(k + 1) * 128, j])
            m[f"convd_{s}"] = cd.astype(BF16)
            m[f"convb_{s}"] = np.asarray(inputs["conv_b"][i], np.float32).reshape(DI, 1).copy()
            m[f"dtb_{s}"] = np.asarray(inputs["dt_proj_b"][i], np.float32).reshape(DI, 1).copy()
            m[f"dvec_{s}"] = np.asarray(inputs["D"][i], np.float32).reshape(DI, 1).copy()
        in_maps.append(m)
    return in_maps


def get_nc(inputs):
    if "nc" not in _CACHE:
        A_log = np.asarray(inputs["A_log"], np.float32)
        A_vals = (-np.exp(A_log[0, 0, :].astype(np.float64))).astype(np.float32)
        _CACHE["nc"] = build_nc(A_vals)
    return _CACHE["nc"]


def kernel(**inputs):
    nc = get_nc(inputs)
    in_maps = make_in_maps(inputs)
    res = run_bass_kernel_spmd(nc, in_maps, core_ids=list(range(NCORES)))
    out = np.zeros((B, H, W, C), np.float32)
    for b in range(B):
        of = res.results[b]["out_full"]
        out[b] = of.T.reshape(H, W, C)
    return out


# revision 24
# speedup vs baseline: 1.0329x; 1.0329x over previous
"""BiMamba2Dv2 Trainium2 kernel (v2).

8 cores = 4 batches x 2 scan directions; each core runs both Mamba stages
for its (batch, dir) in feature-on-partition layout [C|Di, L].

v2 engine mapping (vs baseline):
- selective-scan runs in HALVES (L/2=1152) per (n, block): DVE
  tensor_tensor_scan at ~2.1ns/col with rotating buffers (no drain gaps),
  chained across halves via a [128, 3*16] last-state gather.
- B/C broadcast loads go through the SP hardware DMA queue (frees POOL).
- E=exp(A_n*delta) on ScalarE; delta via one Softplus pass (PSUM src);
  conv as 3 shifted diagonal matmuls on TensorE + Silu on ScalarE;
  z-gate silu on ScalarE straight from PSUM; sz/y stay in SBUF.
- X=du*B on DVE (bf16 2x); hm=h*C split DVE/POOL by state index.
- n-summation via identity matmuls into bank-aligned PSUM tiles
  ([128,1024] x3 + [128,384] tail).
"""

import sys

for _p in ("/opt/trn_rl_repo", "/root/.axon_site/_ro/trn_rl_repo"):
    if _p not in sys.path:
        sys.path.insert(0, _p)

import numpy as np
import ml_dtypes

import concourse.bass as bass
import concourse.bacc as bacc
import concourse.tile as tile
from concourse import mybir
from concourse.bass_utils import run_bass_kernel_spmd

BF16 = ml_dtypes.bfloat16

B, H, W = 4, 48, 48
C = 192
DI = 384
NB = 3             # d-blocks of 128
NST = 16           # state dim
RNK = 12           # dt rank
L = H * W          # 2304
HL = L // 2        # 1152
NCORES = 8
T_TILES = [(0, 512), (512, 512), (1024, 512), (1536, 512), (2048, 256)]
# hm-mul ops on POOL for these states (tunable load-balance knob).
# X-muls feed the DVE scans directly, so they always stay on DVE.
POOL_HM_N = set(range(1, 15))

F32 = mybir.dt.float32
BF = mybir.dt.bfloat16
MUL = mybir.AluOpType.mult
ADD = mybir.AluOpType.add
SUB = mybir.AluOpType.subtract
AFT = mybir.ActivationFunctionType


def _ap(t, free_pairs, off, parts=None):
    part_pair = t.ap[0] if parts is None else parts
    return bass.AP(tensor=t.tensor, offset=t.offset + off, ap=[part_pair] + free_pairs)


def _emit_stage(nc, pools, Wt, u_bf, sfx, A_vals, partial_dram, bc_dram):
    big, med, scr, ps = pools["big"], pools["med"], pools["scr"], pools["ps"]

    w_in = Wt[f"win_{sfx}"]
    w_out = Wt[f"wout_{sfx}"]
    w_xp = Wt[f"wxp_{sfx}"]
    w_dt = Wt[f"wdt_{sfx}"]
    convd = Wt[f"convd_{sfx}"]     # [NB][3] diag matrices on SBUF (bf16)
    convb = Wt[f"convb_{sfx}"]
    dtb = Wt[f"dtb_{sfx}"]
    dvec = Wt[f"dvec_{sfx}"]
    ident = Wt["ident"]

    # ---------------- P1, chunk-major: in_proj / conv / x_proj / dt ----------
    # Per T-chunk emit the full dependency chain so the Tile scheduler can
    # pipeline chunks and start the scan phase before P1 finishes.
    xh = med.tile([128, NB * L], BF, tag="medA", name=f"xh_{sfx}")
    sz = med.tile([128, NB * L], BF, tag="medB", name=f"sz_{sfx}")
    xc = med.tile([128, NB * L], BF, tag="medC", name=f"xc_{sfx}")
    xdbl = scr.tile([12, L], BF, tag="xdbl", name=f"xdbl_{sfx}", bufs=1)
    bcbf = scr.tile([32, L], BF, tag="bcbf", name=f"bcbf_{sfx}", bufs=1)
    delta = big.tile([128, NB * L], F32, tag="bigA", name=f"delta_{sfx}")
    du = med.tile([128, NB * L], BF, tag="medE", name=f"du_{sfx}")

    tags = ["ps", "ps2"]
    tgi = 0

    def _pt(rows=128, cols=512):
        nonlocal tgi
        tgi += 1
        return ps.tile([rows, cols], F32, tag=tags[tgi % 2], name=f"p1_{sfx}{tgi}", bufs=1)

    for ci, (t0, tsz) in enumerate(T_TILES):
        # in_proj: xh rows then z rows
        for m in range(6):
            pt = _pt()
            for k in range(2):
                nc.tensor.matmul(
                    pt[:, :tsz],
                    w_in[k][:, m * 128:(m + 1) * 128],
                    u_bf[k][:, t0:t0 + tsz],
                    start=(k == 0), stop=(k == 1))
            if m < 3:
                nc.scalar.activation(xh[:, m * L + t0: m * L + t0 + tsz],
                                     pt[:, :tsz], AFT.Copy)
            else:
                mm = m - 3
                nc.scalar.activation(sz[:, mm * L + t0: mm * L + t0 + tsz],
                                     pt[:, :tsz], AFT.Silu)
        # conv for this chunk (per block)
        for b in range(NB):
            xb_off = b * L
            pt = _pt()
            nc.tensor.matmul(
                pt[:, :tsz], convd[b][2],
                xh[:, xb_off + t0: xb_off + t0 + tsz],
                start=True, stop=False)
            for j in (1, 0):
                sh = 2 - j
                s0 = t0 - sh
                if s0 >= 0:
                    nc.tensor.matmul(
                        pt[:, :tsz], convd[b][j],
                        xh[:, xb_off + s0: xb_off + s0 + tsz],
                        start=False, stop=(j == 0))
                else:
                    nc.tensor.matmul(
                        pt[:, sh:tsz], convd[b][j],
                        xh[:, xb_off: xb_off + tsz - sh],
                        start=False, stop=False)
            if t0 == 0:
                nc.tensor.matmul(
                    pt[:, :tsz], Wt["zero128"],
                    xh[:, xb_off + t0: xb_off + t0 + tsz],
                    start=False, stop=True)
            nc.scalar.activation(xc[:, xb_off + t0: xb_off + t0 + tsz],
                                 pt[:, :tsz], AFT.Silu, bias=convb[b])
        # x_proj for this chunk
        pt = _pt(12)
        pb = _pt(32)
        for k in range(NB):
            nc.tensor.matmul(
                pt[:, :tsz], w_xp[k][:, 0:RNK],
                xc[:, k * L + t0: k * L + t0 + tsz],
                start=(k == 0), stop=(k == NB - 1))
            nc.tensor.matmul(
                pb[:, :tsz], w_xp[k][:, RNK:44],
                xc[:, k * L + t0: k * L + t0 + tsz],
                start=(k == 0), stop=(k == NB - 1))
        nc.vector.tensor_copy(xdbl[:, t0:t0 + tsz], pt[:, :tsz])
        nc.scalar.activation(bcbf[:, t0:t0 + tsz], pb[:, :tsz], AFT.Copy)
        nc.sync.dma_start(out=bc_dram[:, t0:t0 + tsz], in_=bcbf[:, t0:t0 + tsz])
        # dt_proj + softplus(exp/ln) for this chunk
        for m in range(NB):
            pt = _pt()
            nc.tensor.matmul(
                pt[:, :tsz], w_dt[:, m * 128:(m + 1) * 128],
                xdbl[:, t0:t0 + tsz], start=True, stop=True)
            nc.scalar.activation(delta[:, m * L + t0: m * L + t0 + tsz],
                                 pt[:, :tsz], AFT.Exp, bias=dtb[m])
            nc.scalar.activation(delta[:, m * L + t0: m * L + t0 + tsz],
                                 delta[:, m * L + t0: m * L + t0 + tsz],
                                 AFT.Ln, bias=Wt["ones_col"])
            nc.vector.tensor_tensor(out=du[:, m * L + t0: m * L + t0 + tsz],
                                    in0=delta[:, m * L + t0: m * L + t0 + tsz],
                                    in1=xc[:, m * L + t0: m * L + t0 + tsz], op=MUL)

    # ---------------- P2: selective scan (halves, n inner) ----------------
    yg = med.tile([128, NB * L], BF, tag="medA", name=f"yg_{sfx}")  # reuses xh slot
    hlast = scr.tile([128, NB * NST], BF, tag="hlast", name=f"hl_{sfx}", bufs=1)
    for hf in range(2):
        q0 = hf * HL
        ypb = [pools["ps_big"].tile([128, 1024], F32, tag=f"ypb{b}", name=f"ypb{b}_{sfx}")
               for b in range(NB)]
        ypt = ps.tile([128, NB * 128], F32, tag="ps2", name=f"ypt_{sfx}", bufs=1)
        for n in range(NST):
            Eb = []
            for b in range(NB):
                E = big.tile([128, HL], F32, tag="E", name=f"E{b}_{sfx}", bufs=3)
                nc.scalar.activation(
                    E, delta[:, b * L + q0: b * L + q0 + HL],
                    AFT.Exp, scale=float(A_vals[n]))
                Eb.append(E)
            bcB = scr.tile([128, HL], BF, tag="bcB", name=f"bcB_{sfx}", bufs=2)
            bcC = scr.tile([128, HL], BF, tag="bcC", name=f"bcC_{sfx}", bufs=2)
            nc.sync.dma_start(
                out=bcB, in_=bc_dram.ap()[n:n + 1, q0:q0 + HL].partition_broadcast(128))
            nc.sync.dma_start(
                out=bcC, in_=bc_dram.ap()[NST + n:NST + n + 1, q0:q0 + HL].partition_broadcast(128))
            X = scr.tile([128, NB * HL], BF, tag="X", name=f"X_{sfx}", bufs=2)
            nc.vector.tensor_tensor(
                out=_ap(X, [[HL, NB], [1, HL]], 0),
                in0=_ap(du, [[L, NB], [1, HL]], q0),
                in1=_ap(bcB, [[0, NB], [1, HL]], 0), op=MUL)
            h = scr.tile([128, NB * HL], BF, tag="h", name=f"h_{sfx}", bufs=2)
            for b in range(NB):
                init = 0.0 if hf == 0 else hlast[:, n * NB + b: n * NB + b + 1]
                nc.vector.tensor_tensor_scan(
                    h[:, b * HL:(b + 1) * HL],
                    Eb[b],
                    X[:, b * HL:(b + 1) * HL],
                    init, MUL, ADD)
            if hf == 0:
                nc.vector.tensor_copy(
                    hlast[:, n * NB: n * NB + NB],
                    _ap(h, [[HL, NB], [1, 1]], HL - 1))
            hm = scr.tile([128, NB * HL], BF, tag="X", name=f"hm_{sfx}", bufs=2)
            eng = nc.gpsimd if n in POOL_HM_N else nc.vector
            eng.tensor_tensor(
                out=_ap(hm, [[HL, NB], [1, HL]], 0),
                in0=_ap(h, [[HL, NB], [1, HL]], 0),
                in1=_ap(bcC, [[0, NB], [1, HL]], 0), op=MUL)
            st = (n == 0)
            sp = (n == NST - 1)
            for b in range(NB):
                nc.tensor.matmul(ypb[b][:, 0:512], ident, hm[:, b * HL: b * HL + 512],
                                 start=st, stop=sp)
                nc.tensor.matmul(ypb[b][:, 512:1024], ident, hm[:, b * HL + 512: b * HL + 1024],
                                 start=st, stop=sp)
                nc.tensor.matmul(ypt[:, b * 128:(b + 1) * 128], ident,
                                 hm[:, b * HL + 1024: (b + 1) * HL],
                                 start=st, stop=sp)
        # y = ypsum + xc * D, then gate: yg = y * sz  (per block, this half)
        yh = scr.tile([128, NB * HL], BF, tag="yh", name=f"yh_{sfx}", bufs=1)
        for b in range(NB):
            nc.vector.scalar_tensor_tensor(
                yh[:, b * HL: b * HL + 1024],
                xc[:, b * L + q0: b * L + q0 + 1024],
                dvec[b], ypb[b][:, :], MUL, ADD)
            nc.vector.scalar_tensor_tensor(
                yh[:, b * HL + 1024: (b + 1) * HL],
                xc[:, b * L + q0 + 1024: b * L + q0 + HL],
                dvec[b], ypt[:, b * 128:(b + 1) * 128], MUL, ADD)
            nc.vector.tensor_tensor(out=yg[:, b * L + q0: b * L + q0 + HL],
                                    in0=yh[:, b * HL: (b + 1) * HL],
                                    in1=sz[:, b * L + q0: b * L + q0 + HL], op=MUL)
        # out_proj for the T-chunks fully covered by this half
        for (t0, tsz) in ([(0, 512), (512, 512)] if hf == 0 else
                          [(1024, 512), (1536, 512), (2048, 256)]):
            for m in range(2):
                msz = 128 if m == 0 else 64
                pt = ps.tile([128, 512], F32, tag=tags[(t0 // 512 + m) % 2],
                             name=f"pout_{sfx}", bufs=1)
                for k in range(NB):
                    nc.tensor.matmul(
                        pt[:msz, :tsz],
                        w_out[k][:, m * 128: m * 128 + msz],
                        yg[:, k * L + t0: k * L + t0 + tsz],
                        start=(k == 0), stop=(k == NB - 1))
                stg = scr.tile([128, 512], F32, tag="stg", name=f"stg_{sfx}", bufs=1)
                nc.scalar.activation(stg[:msz, :tsz], pt[:msz, :tsz], AFT.Copy)
                nc.sync.dma_start(out=partial_dram.ap()[m * 128: m * 128 + msz, t0:t0 + tsz],
                                  in_=stg[:msz, :tsz])


def build_nc(A_vals):
    nc = bacc.Bacc("TRN2", target_bir_lowering=False, debug=False,
                   enable_asserts=False, num_devices=NCORES)

    u0_bf = nc.dram_tensor("u0_bf", [C, L], BF, kind="ExternalInput")
    xres = nc.dram_tensor("xres", [C, L], F32, kind="ExternalInput")
    mask = nc.dram_tensor("mask", [128, 1], F32, kind="ExternalInput")
    maskinv = nc.dram_tensor("maskinv", [128, 1], F32, kind="ExternalInput")
    normw = nc.dram_tensor("normw", [C, 1], F32, kind="ExternalInput")
    normb = nc.dram_tensor("normb", [C, 1], F32, kind="ExternalInput")
    ident_in = nc.dram_tensor("ident", [128, 128], BF, kind="ExternalInput")
    wdecl = {}
    for s in ("a", "b"):
        wdecl[f"win_{s}"] = nc.dram_tensor(f"win_{s}", [C, 2 * DI], BF, kind="ExternalInput")
        wdecl[f"wout_{s}"] = nc.dram_tensor(f"wout_{s}", [DI, C], BF, kind="ExternalInput")
        wdecl[f"wxp_{s}"] = nc.dram_tensor(f"wxp_{s}", [DI, 44], BF, kind="ExternalInput")
        wdecl[f"wdt_{s}"] = nc.dram_tensor(f"wdt_{s}", [RNK, DI], BF, kind="ExternalInput")
        wdecl[f"convd_{s}"] = nc.dram_tensor(f"convd_{s}", [NB * 3 * 128, 128], BF, kind="ExternalInput")
        wdecl[f"convb_{s}"] = nc.dram_tensor(f"convb_{s}", [DI, 1], F32, kind="ExternalInput")
        wdecl[f"dtb_{s}"] = nc.dram_tensor(f"dtb_{s}", [DI, 1], F32, kind="ExternalInput")
        wdecl[f"dvec_{s}"] = nc.dram_tensor(f"dvec_{s}", [DI, 1], F32, kind="ExternalInput")
    out_full = nc.dram_tensor("out_full", [C, L], F32, kind="ExternalOutput")

    partial_a = nc.dram_tensor("partial_a", [C, L], F32)
    ssum_a = nc.dram_tensor("ssum_a", [C, L], F32)
    partial_b = nc.dram_tensor("partial_b", [C, L], F32)
    ssum_b = nc.dram_tensor("ssum_b", [C, L], F32)
    bc_dram_a = nc.dram_tensor("bc_dram_a", [32, L], BF)
    bc_dram_b = nc.dram_tensor("bc_dram_b", [32, L], BF)
    stats_dram = nc.dram_tensor("stats_dram", [2, L], F32)

    groups = [[b, b + 4] for b in range(B)]

    import contextlib
    with contextlib.ExitStack() as ctx:
        tc = ctx.enter_context(tile.TileContext(nc))
        pools = {
            "w": ctx.enter_context(tc.tile_pool(name="w", bufs=1)),
            "big": ctx.enter_context(tc.tile_pool(name="big", bufs=1)),
            "med": ctx.enter_context(tc.tile_pool(name="med", bufs=1)),
            "scr": ctx.enter_context(tc.tile_pool(name="scr", bufs=2)),
            "glue": ctx.enter_context(tc.tile_pool(name="glue", bufs=1)),
            "ps": ctx.enter_context(tc.tile_pool(name="ps", bufs=2, space="PSUM")),
            "ps_big": ctx.enter_context(tc.tile_pool(name="ps_big", bufs=1, space="PSUM")),
        }
        wp = pools["w"]

        Wt = {}
        for s in ("a", "b"):
            t1 = wp.tile([128, 2 * DI], BF, tag=f"win0{s}", name=f"win0{s}")
            t2 = wp.tile([64, 2 * DI], BF, tag=f"win1{s}", name=f"win1{s}")
            nc.sync.dma_start(out=t1, in_=wdecl[f"win_{s}"].ap()[0:128, :])
            nc.sync.dma_start(out=t2, in_=wdecl[f"win_{s}"].ap()[128:192, :])
            Wt[f"win_{s}"] = [t1, t2]
            Wt[f"wout_{s}"] = []
            for k in range(NB):
                t = wp.tile([128, C], BF, tag=f"wout{k}{s}", name=f"wout{k}{s}")
                nc.sync.dma_start(out=t, in_=wdecl[f"wout_{s}"].ap()[k * 128:(k + 1) * 128, :])
                Wt[f"wout_{s}"].append(t)
            Wt[f"wxp_{s}"] = []
            for k in range(NB):
                t = wp.tile([128, 44], BF, tag=f"wxp{k}{s}", name=f"wxp{k}{s}")
                nc.sync.dma_start(out=t, in_=wdecl[f"wxp_{s}"].ap()[k * 128:(k + 1) * 128, :])
                Wt[f"wxp_{s}"].append(t)
            t = wp.tile([RNK, DI], BF, tag=f"wdt{s}", name=f"wdt{s}")
            nc.sync.dma_start(out=t, in_=wdecl[f"wdt_{s}"].ap()[:, :])
            Wt[f"wdt_{s}"] = t
            # conv diag matrices [NB][3] each [128,128] bf16
            Wt[f"convd_{s}"] = []
            for k in range(NB):
                taps = []
                for j in range(3):
                    t = wp.tile([128, 128], BF, tag=f"cvd{k}{j}{s}", name=f"cvd{k}{j}{s}")
                    off = (k * 3 + j) * 128
                    nc.sync.dma_start(out=t, in_=wdecl[f"convd_{s}"].ap()[off:off + 128, :])
                    taps.append(t)
                Wt[f"convd_{s}"].append(taps)
            for nm in ("convb", "dtb", "dvec"):
                lst = []
                for k in range(NB):
                    t = wp.tile([128, 1], F32, tag=f"{nm}{k}{s}", name=f"{nm}{k}{s}")
                    nc.sync.dma_start(out=t, in_=wdecl[f"{nm}_{s}"].ap()[k * 128:(k + 1) * 128, :])
                    lst.append(t)
                Wt[f"{nm}_{s}"] = lst
        idt = wp.tile([128, 128], BF, tag="ident", name="ident_t")
        nc.sync.dma_start(out=idt, in_=ident_in.ap()[:, :])
        Wt["ident"] = idt
        z128 = wp.tile([128, 128], BF, tag="zero128", name="zero128")
        nc.vector.memset(z128, 0.0)
        Wt["zero128"] = z128
        nw = [wp.tile([128, 1], F32, tag="nw0", name="nw0"),
              wp.tile([64, 1], F32, tag="nw1", name="nw1")]
        nb_ = [wp.tile([128, 1], F32, tag="nb0", name="nb0"),
               wp.tile([64, 1], F32, tag="nb1", name="nb1")]
        nc.sync.dma_start(out=nw[0], in_=normw.ap()[0:128, :])
        nc.sync.dma_start(out=nw[1], in_=normw.ap()[128:192, :])
        nc.sync.dma_start(out=nb_[0], in_=normb.ap()[0:128, :])
        nc.sync.dma_start(out=nb_[1], in_=normb.ap()[128:192, :])
        msk = wp.tile([128, 1], F32, tag="msk", name="msk")
        mskv = wp.tile([128, 1], F32, tag="mskv", name="mskv")
        nc.sync.dma_start(out=msk, in_=mask.ap()[:, :])
        nc.sync.dma_start(out=mskv, in_=maskinv.ap()[:, :])
        ones_a = wp.tile([128, 1], F32, tag="ones_a", name="ones_a")
        ones_b = wp.tile([64, 1], F32, tag="ones_b", name="ones_b")
        nc.vector.memset(ones_a, 1.0)
        nc.vector.memset(ones_b, 1.0)
        ones_abf = wp.tile([128, 1], BF, tag="ones_abf", name="ones_abf")
        ones_bbf = wp.tile([64, 1], BF, tag="ones_bbf", name="ones_bbf")
        nc.vector.memset(ones_abf, 1.0)
        nc.vector.memset(ones_bbf, 1.0)
        Wt["ones_col"] = ones_a

        uA = [wp.tile([128, L], BF, tag="uin0", name="uA0"),
              wp.tile([64, L], BF, tag="uin1", name="uA1")]
        nc.sync.dma_start(out=uA[0], in_=u0_bf.ap()[0:128, :])
        nc.sync.dma_start(out=uA[1], in_=u0_bf.ap()[128:192, :])

        _emit_stage(nc, pools, Wt, uA, "a", A_vals, partial_a, bc_dram_a)

        nc.gpsimd.collective_compute(
            "AllReduce", ADD, replica_groups=groups,
            ins=[partial_a.ap().opt()], outs=[ssum_a.ap().opt()])

        # ---------------- glue ----------------
        gl = pools["glue"]
        big = pools["big"]
        med = pools["med"]
        # packed [128, 2L]: cols 0:L = channels 0..127, cols L:2L (rows 0:64) = 128..191
        st = big.tile([128, 2 * L], BF, tag="bigB", name="st_g")
        fl = big.tile([128, 2 * L], BF, tag="bigC", name="fl_g")
        res = med.tile([128, 2 * L], BF, tag="medB", name="res_g")
        sq = big.tile([128, 2 * L], F32, tag="bigA", name="sq_g")
        epst = gl.tile([1, 1], F32, tag="epst", name="epst_g")
        ssb = med.tile([128, 2 * L], BF, tag="medA", name="ssb_g")
        nc.gpsimd.dma_start(out=ssb[:, 0:L], in_=ssum_a.ap()[0:128, :])
        nc.gpsimd.dma_start(out=ssb[0:64, L:2 * L], in_=ssum_a.ap()[128:192, :])
        for p in range(2):
            psz = 128 if p == 0 else 64
            co = p * L
            # permuted straight view (DVE) & flipped view (POOL) in parallel
            nc.vector.tensor_copy(
                _ap(st, [[48, 48], [1, 48]], co, parts=[st.ap[0][0], psz]),
                _ap(ssb, [[1, 48], [48, 48]], co, parts=[ssb.ap[0][0], psz]))
            nc.gpsimd.tensor_copy(
                _ap(fl, [[48, 48], [1, 48]], co, parts=[fl.ap[0][0], psz]),
                _ap(ssb, [[-1, 48], [-48, 48]], co + L - 1, parts=[ssb.ap[0][0], psz]))
            nc.gpsimd.dma_start(out=res[0:psz, co:co + L], in_=xres.ap()[p * 128:p * 128 + psz, :])
            # select: st = st*maskinv + fl*mask
            nc.vector.tensor_scalar(out=fl[0:psz, co:co + L], in0=fl[0:psz, co:co + L],
                                    scalar1=msk[:psz, :], scalar2=None, op0=MUL)
            nc.vector.scalar_tensor_tensor(
                st[0:psz, co:co + L], st[0:psz, co:co + L], mskv[:psz, :],
                fl[0:psz, co:co + L], MUL, ADD)

        # pass 1: mean over channels via ones-matmul (chunked drains)
        for (t0, tsz) in T_TILES:
            p1 = pools["ps"].tile([1, 512], F32, tag="ps", name="lnp1", bufs=1)
            for p in range(2):
                one = ones_abf if p == 0 else ones_bbf
                nc.tensor.matmul(p1[:, :tsz], one,
                                 st[0:(128 if p == 0 else 64), p * L + t0: p * L + t0 + tsz],
                                 start=(p == 0), stop=(p == 1))
            rA = gl.tile([1, 512], F32, tag="rA", name="rA_g", bufs=2)
            nc.vector.tensor_scalar(out=rA[:, :tsz], in0=p1[:, :tsz],
                                    scalar1=1.0 / C, scalar2=None, op0=MUL)
            nc.sync.dma_start(out=stats_dram[0:1, t0:t0 + tsz], in_=rA[:, :tsz])
        mbc = big.tile([128, L], F32, tag="bigC", name="mbc_g")
        nc.sync.dma_start(out=mbc, in_=stats_dram.ap()[0:1, :].partition_broadcast(128))
        # center x, square, pass 2: variance
        for p in range(2):
            psz = 128 if p == 0 else 64
            co = p * L
            nc.vector.tensor_tensor(out=st[0:psz, co:co + L], in0=st[0:psz, co:co + L],
                                    in1=mbc[0:psz, :], op=SUB)
            nc.scalar.activation(sq[0:psz, co:co + L], st[0:psz, co:co + L], AFT.Square)
        nc.vector.memset(epst, 1e-5)
        for (t0, tsz) in T_TILES:
            p2 = pools["ps"].tile([1, 512], F32, tag="ps", name="lnp2", bufs=1)
            for p in range(2):
                one = ones_a if p == 0 else ones_b
                nc.tensor.matmul(p2[:, :tsz], one,
                                 sq[0:(128 if p == 0 else 64), p * L + t0: p * L + t0 + tsz],
                                 start=(p == 0), stop=(p == 1))
            rB = gl.tile([1, 512], F32, tag="rA", name="rB_g", bufs=2)
            nc.vector.tensor_scalar(out=rB[:, :tsz], in0=p2[:, :tsz],
                                    scalar1=1.0 / C, scalar2=None, op0=MUL)
            nc.scalar.activation(rB[:, :tsz], rB[:, :tsz], AFT.Sqrt, bias=epst)
            nc.vector.reciprocal(rB[:, :tsz], rB[:, :tsz])
            nc.sync.dma_start(out=stats_dram[1:2, t0:t0 + tsz], in_=rB[:, :tsz])
        rbc = big.tile([128, L], F32, tag="bigA", name="rbc_g")
        nc.sync.dma_start(out=rbc, in_=stats_dram.ap()[1:2, :].partition_broadcast(128))
        uB = [wp.tile([128, L], BF, tag="uin0", name="uB0"),
              wp.tile([64, L], BF, tag="uin1", name="uB1")]
        for p in range(2):
            psz = 128 if p == 0 else 64
            co = p * L
            sl = st[0:psz, co:co + L]
            nc.vector.tensor_tensor(out=sl, in0=sl, in1=rbc[0:psz, :], op=MUL)
            nc.vector.scalar_tensor_tensor(sl, sl, nw[p], res[0:psz, co:co + L], MUL, ADD)
            nc.vector.tensor_scalar(out=sl, in0=sl, scalar1=nb_[p], scalar2=None, op0=ADD)
            nc.vector.tensor_copy(uB[p], sl)

        _emit_stage(nc, pools, Wt, uB, "b", A_vals, partial_b, bc_dram_b)

        nc.gpsimd.collective_compute(
            "AllReduce", ADD, replica_groups=groups,
            ins=[partial_b.ap().opt()], outs=[ssum_b.ap().opt()])
        nc.sync.dma_start(out=out_full[:, :], in_=ssum_b.ap()[:, :])

    nc.compile()
    return nc


_CACHE = {}


def make_in_maps(inputs):
    x = np.asarray(inputs["x"], np.float32)
    in_maps = []
    for core in range(NCORES):
        b, dr = core % 4, core // 4
        xw = x[b].transpose(1, 0, 2).reshape(L, C).T.copy()
        xh_ = x[b].reshape(L, C).T.copy()
        if dr == 1:
            xw = xw[:, ::-1].copy()
            xh_ = xh_[:, ::-1].copy()
        m = {
            "u0_bf": xw.astype(BF16),
            "xres": xh_.astype(np.float32),
            "mask": np.full((128, 1), float(dr), np.float32),
            "maskinv": np.full((128, 1), 1.0 - float(dr), np.float32),
            "normw": np.asarray(inputs["norm_w"], np.float32).reshape(C, 1).copy(),
            "normb": np.asarray(inputs["norm_b"], np.float32).reshape(C, 1).copy(),
            "ident": np.eye(128, dtype=BF16),
        }
        for s, i in (("a", dr), ("b", 2 + dr)):
            m[f"win_{s}"] = np.asarray(inputs["in_proj_w"][i], np.float32).T.copy().astype(BF16)
            m[f"wout_{s}"] = np.asarray(inputs["out_proj_w"][i], np.float32).T.copy().astype(BF16)
            m[f"wxp_{s}"] = np.asarray(inputs["x_proj_w"][i], np.float32).T.copy().astype(BF16)
            m[f"wdt_{s}"] = np.asarray(inputs["dt_proj_w"][i], np.float32).T.copy().astype(BF16)
            cw = np.asarray(inputs["conv_w"][i], np.float32)  # [DI, 3]
            cd = np.zeros((NB * 3 * 128, 128), np.float32)
            for k in range(NB):
                for j in range(3):
                    off = (k * 3 + j) * 128
                    cd[off:off + 128, :] = np.diag(cw[k * 128:(k + 1) * 128, j])
            m[f"convd_{s}"] = cd.astype(BF16)
            m[f"convb_{s}"] = np.asarray(inputs["conv_b"][i], np.float32).reshape(DI, 1).copy()
            m[f"dtb_{s}"] = np.asarray(inputs["dt_proj_b"][i], np.float32).reshape(DI, 1).copy()
            m[f"dvec_{s}"] = np.asarray(inputs["D"][i], np.float32).reshape(DI, 1).copy()
        in_maps.append(m)
    return in_maps


def get_nc(inputs):
    if "nc" not in _CACHE:
        A_log = np.asarray(inputs["A_log"], np.float32)
        A_vals = (-np.exp(A_log[0, 0, :].astype(np.float64))).astype(np.float32)
        _CACHE["nc"] = build_nc(A_vals)
    return _CACHE["nc"]


def kernel(**inputs):
    nc = get_nc(inputs)
    in_maps = make_in_maps(inputs)
    res = run_bass_kernel_spmd(nc, in_maps, core_ids=list(range(NCORES)))
    out = np.zeros((B, H, W, C), np.float32)
    for b in range(B):
        of = res.results[b]["out_full"]
        out[b] = of.T.reshape(H, W, C)
    return out
